# revision 1
# baseline (speedup 1.0000x reference)
"""GCN (3x GCNConv + BN + ReLU, global mean pool, linear) on 8 Trainium2 cores.

Self-contained: hardcodes all shapes. Strategy:
  - Nodes block-sharded across 8 cores (12500 each); edges partitioned by dst.
  - Per layer: local GEMM h@W (bf16 on PE), row-scale by deg^-1/2, AllGather the
    bf16 node table in 4 node-slices (so gather indices fit int16), then each
    core dma_gathers its edges' source rows and segment-sums them with 0/1
    fp8 selector matmuls into fp32 PSUM (128-dst blocks).
  - BatchNorm(+bias)+ReLU folded into per-feature constants on host; per-node
    deg^-1/2 applied with per-partition tensor_scalar ops.
  - Pooling: one-hot matmul accumulates [128f, 512g] partial sums, AllReduce
    across cores, final linear on device.
"""
import os
import numpy as np
import ml_dtypes

F = 128
P = 8
B_PIECE = 12
EPS = np.float32(1e-5)


def _set_sizes(n, e, g):
    global N, E, G, NSH, NB, NPAD, TPS, SLICE_TILES, SLICE_ROWS, TBL_ROWS, PIECES
    N, E, G = n, e, g
    NSH = N // P
    NB = -(-NSH // 128)
    NPAD = NB * 128
    TPS = -(-NB // 4)
    SLICE_TILES = [TPS, TPS, TPS, NB - 3 * TPS]
    assert SLICE_TILES[3] > 0
    SLICE_ROWS = [t * 128 for t in SLICE_TILES]
    TBL_ROWS = [P * r for r in SLICE_ROWS]
    assert max(TBL_ROWS) < 32768, "gather idx must fit int16"
    PIECES = [(i, min(i + B_PIECE, NB)) for i in range(0, NB, B_PIECE)]


_set_sizes(100000, 1600000, 512)

_MAXK = {0: 128, 32: 32, 64: 64}

_LAST_RESULTS = {}  # stash for test harness (exec time etc.)


def _build_schedule(L):
    """L: [4][NB] int array of 32-multiple group lengths (same on all cores).

    Returns (pieces, NPOS) where pieces is a list over (k, piece) of dicts:
      k, pos0, npos, blocks: list of (b, segs) with segs = [(col, off, K), ...]
    Positions are global across the whole (k, piece) ordering.
    """
    pieces = []
    pos = 0
    for k in range(4):
        for (b0, b1) in PIECES:
            pstart = pos
            blocks = []
            q = 0  # position relative to piece start
            for b in range(b0, b1):
                r = int(L[k][b])
                if r > 0 and q % 128 == 96:
                    q += 32  # group starts at partition 96 are illegal
                segs = []
                while r > 0:
                    off = q % 128
                    K = min(r, _MAXK[off], 128 - off)
                    segs.append((q // 128, off, K))
                    q += K
                    r -= K
                if segs:
                    blocks.append((b, segs))
            npos = (q + 127) // 128 * 128
            pieces.append(dict(k=k, pos0=pstart, npos=npos, blocks=blocks))
            pos += npos
    return pieces, pos


def _preprocess(x, edge_index, batch, Ws, c2s, Wl, bl):
    """Build per-core device inputs. Ws: 3 pre-folded [128,128] f32 weights;
    c2s: 3 [128] f32 epilogue biases; Wl [128,1] f32; bl scalar f32."""
    src = np.asarray(edge_index[0], dtype=np.int64)
    dst = np.asarray(edge_index[1], dtype=np.int64)
    loops = np.arange(N, dtype=np.int64)
    src = np.concatenate([src, loops])
    dst = np.concatenate([dst, loops])

    deg = np.bincount(dst, minlength=N).astype(np.float32)

    # relabel nodes: sort by in-degree, deal round-robin across cores so each
    # (core, block) sees a near-identical degree profile -> the max-over-cores
    # group padding collapses toward the mean.
    order = np.argsort(-deg, kind="stable")
    perm = np.empty(N, np.int64)  # old id -> new id
    ranks = np.arange(N)
    perm[order] = (ranks % P) * NSH + ranks // P
    src = perm[src]
    dst = perm[dst]
    inv = np.empty(N, np.int64)   # new id -> old id
    inv[perm] = np.arange(N)
    x = x[inv]
    batch = np.asarray(batch, np.int64)[inv]
    deg = deg[inv]

    dinv = (1.0 / np.sqrt(np.maximum(deg, 1.0))).astype(np.float32)

    # src -> (slice k, table row)
    so = src // NSH
    si = src % NSH
    sk = np.minimum(si // SLICE_ROWS[0], 3)
    srow = so * np.array(SLICE_ROWS, np.int64)[sk] + (si - sk * SLICE_ROWS[0])
    assert srow.max() < max(TBL_ROWS)

    core = dst // NSH
    dl = dst % NSH
    db = dl // 128
    dcol = dl % 128

    # dedup: within a (core, k, block) group, a source row gathered once can
    # feed several edges via selector multiplicity. Count distinct rows.
    ekey = ((core * 4 + sk) * NB + db) * np.int64(32768) + srow
    uniq = np.unique(ekey)
    ug = uniq // 32768
    cnt = np.bincount(ug, minlength=P * 4 * NB).reshape(P, 4, NB)
    L = cnt.max(axis=0)
    L = (L + 63) // 64 * 64
    pieces, NPOS = _build_schedule(L)

    # group start positions (global)
    gstart = np.zeros((4, NB), np.int64)
    for pc in pieces:
        k = pc["k"]
        q = pc["pos0"]
        for b, _segs in pc["blocks"]:
            if q % 128 == 96:
                q += 32
            gstart[k][b] = q
            q += L[k][b]
        # blocks with L==0 within the piece get no slot (none exist: every
        # block has self-loops, but guard anyway)

    per_core = []
    for c in range(P):
        m = core == c
        skc, dbc, dcolc, srowc = sk[m], db[m], dcol[m], srow[m]
        # sort edges by (k, b, srow); dedup rows within each group
        order = np.lexsort((srowc, dbc, skc))
        skc, dbc, dcolc, srowc = (a[order] for a in (skc, dbc, dcolc, srowc))
        gid = (skc * NB + dbc) * np.int64(32768) + srowc
        first = np.r_[True, gid[1:] != gid[:-1]]          # first edge of a row
        urank = np.cumsum(first) - 1                      # dedup'd row index
        ggid = skc * NB + dbc
        gfirstmask = np.r_[True, ggid[1:] != ggid[:-1]]   # first edge of group
        # dedup'd rank within group: urank - urank[group start]
        gstart_urank = urank[gfirstmask]
        gsz = np.diff(np.r_[np.flatnonzero(gfirstmask), ggid.size])
        rank = urank - np.repeat(gstart_urank, gsz)
        posn = gstart[skc, dbc] + rank
        idx_flat = np.zeros(NPOS, np.int16)
        idx_flat[posn] = srowc.astype(np.int16)
        sel = np.zeros((NPOS, 128), np.float32)
        np.add.at(sel, (posn, dcolc), 1.0)
        sel8 = sel.astype(ml_dtypes.float8_e4m3)
        assert float(sel.max()) <= 240.0
        idx_t = np.tile(idx_flat.reshape(NPOS // 16, 16).T, (8, 1)).copy()

        # node-local data
        lo = c * NSH
        xT = np.zeros((F, NPAD), np.float32)
        xT[:, :NSH] = x[lo:lo + NSH].T
        dv = np.zeros(NPAD, np.float32)
        dv[:NSH] = dinv[lo:lo + NSH]
        dinv_t = dv.reshape(NB, 128).T.copy()
        bv = np.full(NPAD, -1.0, np.float32)
        bv[:NSH] = np.asarray(batch[lo:lo + NSH], dtype=np.int64).astype(np.float32)
        batch_t = bv.reshape(NB, 128).T.copy()

        per_core.append(dict(
            xT=xT, dinv=dinv_t, idx=idx_t, sel=sel8, pool=batch_t,
        ))

    # shared constants
    cnt_g = np.bincount(np.asarray(batch, np.int64), minlength=G).astype(np.float32)
    invcnt = (1.0 / np.maximum(cnt_g, 1.0)).astype(np.float32)
    Wcat = np.concatenate([w.astype(np.float32) for w in Ws], axis=1).astype(np.float32)  # [128, 384]
    c2cat = np.concatenate([np.tile(c2[None, :], (128, 1)) for c2 in c2s], axis=1).astype(np.float32)  # [128, 384]
    post = np.stack([invcnt, np.full(G, np.float32(bl))]).astype(np.float32)  # [2, 512]
    ident = np.eye(128, dtype=np.float32)
    iota = np.tile(np.arange(G, dtype=np.float32)[None, :], (128, 1))

    shared = dict(W=Wcat, c2=c2cat, post=post, ident=ident, iota=iota,
                  Wl=Wl.astype(np.float32).reshape(128, 1))
    return per_core, shared, pieces, NPOS, L


def _build_bass(pieces, NPOS, no_collectives=False):
    import concourse.bacc as bacc
    import concourse.mybir as mybir
    from concourse.tile import TileContext, add_dep_helper

    no_gather = bool(int(os.environ.get("GCN_NO_GATHER", "0")))
    no_sel = bool(int(os.environ.get("GCN_NO_SEL", "0")))
    no_mm = bool(int(os.environ.get("GCN_NO_MM", "0")))
    gsplit = int(os.environ.get("GCN_GATHER_SPLIT", "1024"))
    stages = int(os.environ.get("GCN_STAGES", "6"))
    # stages: 1=GEMM only, 2=+AG, 3=+gather, 4=+selector matmuls,
    #         5=+epilogue, 6=full (pool+final)

    nc = bacc.Bacc("TRN2", target_bir_lowering=False, debug=False)
    dt = mybir.dt

    xT_in = nc.dram_tensor("xT", [F, NPAD], dt.float32, kind="ExternalInput")
    dinv_in = nc.dram_tensor("dinv", [128, NB], dt.float32, kind="ExternalInput")
    idx_in = nc.dram_tensor("idx", [128, NPOS // 16], dt.int16, kind="ExternalInput")
    sel_in = nc.dram_tensor("sel", [NPOS, 128], dt.float8e4, kind="ExternalInput")
    pool_in = nc.dram_tensor("pool", [128, NB], dt.float32, kind="ExternalInput")
    iota_in = nc.dram_tensor("iota", [128, G], dt.float32, kind="ExternalInput")
    W_in = nc.dram_tensor("W", [128, 384], dt.float32, kind="ExternalInput")
    c2_in = nc.dram_tensor("c2", [128, 384], dt.float32, kind="ExternalInput")
    post_in = nc.dram_tensor("post", [2, G], dt.float32, kind="ExternalInput")
    ident_in = nc.dram_tensor("ident", [128, 128], dt.float32, kind="ExternalInput")
    Wl_in = nc.dram_tensor("Wl", [128, 1], dt.float32, kind="ExternalInput")

    out_d = nc.dram_tensor("out", [1, G], dt.float32, kind="ExternalOutput")

    # internal DRAM: double-buffered per-parity cc inputs and tables
    cc_ins = [[nc.dram_tensor(f"ccin_{p}_{k}", [SLICE_ROWS[k], F], dt.bfloat16)
               for k in range(4)] for p in range(2)]
    tables = [[nc.dram_tensor(f"tbl_{p}_{k}", [TBL_ROWS[k], F], dt.bfloat16,
                              addr_space="Shared") for k in range(4)] for p in range(2)]
    ar_in = nc.dram_tensor("ar_in", [128, G], dt.float32)
    ar_out = nc.dram_tensor("ar_out", [128, G], dt.float32, addr_space="Shared")

    rg = [list(range(P))]
    maxc = max(pc["npos"] for pc in pieces) // 128

    with TileContext(nc) as tc:
        with (
            tc.tile_pool(name="const", bufs=1) as cst,
            tc.tile_pool(name="big", bufs=1) as big,
            tc.tile_pool(name="io", bufs=int(os.environ.get("GCN_IO_BUFS", "3"))) as io,
            tc.tile_pool(name="stream", bufs=int(os.environ.get("GCN_STRM_BUFS", "2"))) as strm,
            tc.tile_pool(name="pgemm", bufs=2, space="PSUM") as pgemm,
            tc.tile_pool(name="ptrans", bufs=1, space="PSUM") as ptrans,
            tc.tile_pool(name="ppart", bufs=3, space="PSUM") as ppart,
            tc.tile_pool(name="ppool", bufs=1, space="PSUM") as ppool,
            tc.tile_pool(name="pfin", bufs=1, space="PSUM") as pfin,
        ):
            # constants
            W_sb = cst.tile([128, 384], dt.float32, tag="W")
            nc.sync.dma_start(out=W_sb[:, :], in_=W_in[:, :])
            c2_sb = cst.tile([128, 384], dt.float32, tag="c2")
            nc.sync.dma_start(out=c2_sb[:, :], in_=c2_in[:, :])
            dinv_sb = cst.tile([128, NB], dt.float32, tag="dinv")
            nc.sync.dma_start(out=dinv_sb[:, :], in_=dinv_in[:, :])
            ident_sb = cst.tile([128, 128], dt.float32, tag="ident")
            nc.sync.dma_start(out=ident_sb[:, :], in_=ident_in[:, :])
            Wl_sb = cst.tile([128, 1], dt.float32, tag="Wl")
            nc.sync.dma_start(out=Wl_sb[:, :], in_=Wl_in[:, :])
            iota_sb = cst.tile([128, G], dt.float32, tag="iota")
            nc.sync.dma_start(out=iota_sb[:, :], in_=iota_in[:, :])
            batch_sb = cst.tile([128, NB], dt.float32, tag="batchv")
            nc.sync.dma_start(out=batch_sb[:, :], in_=pool_in[:, :])
            invcnt_sb = cst.tile([1, G], dt.float32, tag="invcnt")
            nc.sync.dma_start(out=invcnt_sb[:, :], in_=post_in[0:1, :])
            blrow_sb = cst.tile([1, G], dt.float32, tag="blrow")
            nc.sync.dma_start(out=blrow_sb[:, :], in_=post_in[1:2, :])

            z_prev = None
            ag_by_parity = {}        # parity -> {k: ag inst}
            gathers_by_parity = {}   # parity -> {k: [gather insts]}
            poolT_ps = ppool.tile([128, G], dt.float32, tag="poolT")

            for l in range(3):
                par = l % 2
                # ---------------- GEMM phase ----------------
                ag_insts = {}
                for b in range(NB):
                    if l == 0:
                        lhsT = io.tile([128, 128], dt.float32, tag="lhsT")
                        nc.sync.dma_start(out=lhsT[:, :], in_=xT_in[:, b * 128:(b + 1) * 128])
                    else:
                        zT_ps = ptrans.tile([128, 128], dt.float32, tag="zT")
                        nc.tensor.transpose(zT_ps[:, :], z_prev[:, b * 128:(b + 1) * 128], ident_sb[:, :])
                        lhsT = io.tile([128, 128], dt.float32, tag="lhsT")
                        nc.vector.tensor_copy(lhsT[:, :], zT_ps[:, :])
                    t_ps = pgemm.tile([128, 128], dt.float32, tag="t")
                    nc.tensor.matmul(t_ps[:, :], lhsT=lhsT[:, :], rhs=W_sb[:, l * 128:(l + 1) * 128],
                                     start=True, stop=True)
                    hp = io.tile([128, 128], dt.bfloat16, tag="hp")
                    nc.vector.tensor_scalar_mul(hp[:, :], t_ps[:, :], dinv_sb[:, b:b + 1])
                    k = b // TPS
                    roff = (b - k * TPS) * 128
                    d = nc.sync.dma_start(out=cc_ins[par][k][roff:roff + 128, :], in_=hp[:, :])
                    if par in ag_by_parity:
                        add_dep_helper(d.ins, ag_by_parity[par][k].ins,
                                       reason="ccin WAR vs prev AG same parity")
                    if (b % TPS == TPS - 1 or b == NB - 1) and stages >= 2:
                        if no_collectives:
                            # timing stand-in: local copy of own shard
                            ag = nc.sync.dma_start(
                                out=tables[par][k][0:SLICE_ROWS[k], :],
                                in_=cc_ins[par][k][:, :])
                        else:
                            ag = nc.gpsimd.collective_compute(
                                "AllGather", mybir.AluOpType.bypass, replica_groups=rg,
                                ins=[cc_ins[par][k].ap().opt()],
                                outs=[tables[par][k].ap().opt()],
                            )
                        # WAR: table written by this AG was read by gathers 2 layers ago
                        for gi in gathers_by_parity.get(par, {}).get(k, []):
                            add_dep_helper(ag.ins, gi.ins, reason="table WAR vs old gathers")
                        ag_insts[k] = ag
                ag_by_parity[par] = ag_insts
                gathers_by_parity[par] = {k: [] for k in range(4)}

                # ---------------- SpMM phase ----------------
                if stages < 3:
                    break
                agg = big.tile([128, NPAD], dt.float32, tag="agg")
                first_k = {}
                for pc in pieces:
                    k = pc["k"]
                    npos = pc["npos"]
                    cols = npos // 128
                    idxt = strm.tile([128, maxc * 8], dt.int16, tag="idx")
                    nc.sync.dma_start(out=idxt[:, :npos // 16],
                                      in_=idx_in[:, pc["pos0"] // 16:(pc["pos0"] + npos) // 16])
                    msgt = strm.tile([128, maxc, 128], dt.bfloat16, tag="msg")
                    if no_gather:
                        nc.vector.memset(msgt[:, :cols, :], 0.0)
                    elif gsplit:
                        for s0 in range(0, npos, gsplit):
                            ns = min(gsplit, npos - s0)
                            g = nc.gpsimd.dma_gather(
                                msgt[:, s0 // 128:(s0 + ns) // 128, :],
                                tables[par][k][:, :],
                                idxt[:, s0 // 16:(s0 + ns) // 16],
                                ns, ns, F,
                            )
                            add_dep_helper(g.ins, ag_insts[k].ins, reason="gather RAW on AG")
                            gathers_by_parity[par][k].append(g)
                    else:
                        g = nc.gpsimd.dma_gather(
                            msgt[:, :cols, :], tables[par][k][:, :], idxt[:, :npos // 16],
                            npos, npos, F,
                        )
                        add_dep_helper(g.ins, ag_insts[k].ins, reason="gather RAW on AG")
                        gathers_by_parity[par][k].append(g)
                    selt = strm.tile([128, maxc, 128], dt.float8e4, tag="sel")
                    if not no_sel:
                        nc.sync.dma_start(
                            out=selt[:, :cols, :],
                            in_=sel_in[pc["pos0"]:pc["pos0"] + npos, :].rearrange("(c p) d -> p c d", p=128),
                        )
                    for b, segs in (pc["blocks"] if stages >= 4 else []):
                        ps = ppart.tile([128, 128], dt.float32, tag="part")
                        if no_mm:
                            nc.vector.memset(ps[:, :], 0.0)
                            segs = []
                        for i, (col, off, K) in enumerate(segs):
                            nc.tensor.matmul(
                                ps[:, :],
                                lhsT=selt[off:off + K, col, :],
                                rhs=msgt[off:off + K, col, :],
                                start=(i == 0), stop=(i == len(segs) - 1),
                            )
                        sl = agg[:, b * 128:(b + 1) * 128]
                        if b not in first_k:
                            first_k[b] = k
                            nc.vector.tensor_copy(sl, ps[:, :])
                        else:
                            nc.vector.tensor_add(sl, sl, ps[:, :])

                # ---------------- epilogue ----------------
                if stages < 5:
                    break
                z = big.tile([128, NPAD], dt.float32, tag="z")
                for b in range(NB):
                    sl = agg[:, b * 128:(b + 1) * 128]
                    v = io.tile([128, 128], dt.float32, tag="v")
                    nc.vector.scalar_tensor_tensor(
                        v[:, :], sl, dinv_sb[:, b:b + 1], c2_sb[:, l * 128:(l + 1) * 128],
                        op0=mybir.AluOpType.mult, op1=mybir.AluOpType.add,
                    )
                    zsl = z[:, b * 128:(b + 1) * 128]
                    if int(os.environ.get("GCN_RELU_DVE", "1")):
                        nc.vector.tensor_scalar_max(zsl, v[:, :], 0.0)
                    else:
                        nc.scalar.activation(zsl, v[:, :], mybir.ActivationFunctionType.Relu)
                    if l == 2 and stages >= 6:
                        pst = strm.tile([128, G], dt.float32, tag="poolsel")
                        nc.vector.tensor_scalar(
                            pst[:, :], iota_sb[:, :], batch_sb[:, b:b + 1], None,
                            op0=mybir.AluOpType.is_equal,
                        )
                        nc.tensor.matmul(poolT_ps[:, :], lhsT=zsl, rhs=pst[:, :],
                                         start=(b == 0), stop=(b == NB - 1))
                z_prev = z

            # ---------------- pooling reduce + final linear ----------------
            if stages < 6:
                dummy = cst.tile([1, G], dt.float32, tag="dummy")
                nc.vector.memset(dummy[:, :], 0.0)
                nc.sync.dma_start(out=out_d[:, :], in_=dummy[:, :])
            else:
                poolT_sb = cst.tile([128, G], dt.float32, tag="poolTsb")
                nc.vector.tensor_copy(poolT_sb[:, :], poolT_ps[:, :])
                nc.sync.dma_start(out=ar_in[:, :], in_=poolT_sb[:, :])
                if no_collectives:
                    ar = nc.sync.dma_start(out=ar_out[:, :], in_=ar_in[:, :])
                else:
                    ar = nc.gpsimd.collective_compute(
                        "AllReduce", mybir.AluOpType.add, replica_groups=rg,
                        ins=[ar_in.ap().opt()], outs=[ar_out.ap().opt()],
                    )
                poolF = cst.tile([128, G], dt.float32, tag="poolF")
                d = nc.sync.dma_start(out=poolF[:, :], in_=ar_out[:, :])
                add_dep_helper(d.ins, ar.ins, reason="read AR output")
                out_ps = pfin.tile([1, G], dt.float32, tag="fin")
                nc.tensor.matmul(out_ps[:, :], lhsT=Wl_sb[:, :], rhs=poolF[:, :],
                                 start=True, stop=True)
                orow = cst.tile([1, G], dt.float32, tag="orow")
                nc.vector.tensor_mul(orow[:, :], out_ps[:, :], invcnt_sb[:, :])
                nc.vector.tensor_add(orow[:, :], orow[:, :], blrow_sb[:, :])
                nc.sync.dma_start(out=out_d[:, :], in_=orow[:, :])

    nc.compile()
    return nc


def kernel(x, edge_index, batch,
           W1, b1, g1, be1, m1, v1,
           W2, b2, g2, be2, m2, v2,
           W3, b3, g3, be3, m3, v3,
           Wl, bl):
    from concourse.bass_utils import run_bass_kernel_spmd

    x = np.asarray(x, np.float32)
    # fold BN into per-feature scale s1 (>0) and epilogue bias c2
    Ws, c2s = [], []
    prev_s1 = None
    for (W, b, g, be, m, v) in [(W1, b1, g1, be1, m1, v1),
                                (W2, b2, g2, be2, m2, v2),
                                (W3, b3, g3, be3, m3, v3)]:
        W = np.asarray(W, np.float32)
        b = np.asarray(b, np.float32)
        g = np.asarray(g, np.float32)
        be = np.asarray(be, np.float32)
        m = np.asarray(m, np.float32)
        v = np.asarray(v, np.float32)
        s1 = g / np.sqrt(v + EPS)
        assert np.all(s1 > 0), "BN scale must be positive for ReLU folding"
        s2 = be - m * s1
        c2 = b + s2 / s1
        if prev_s1 is not None:
            W = prev_s1[:, None] * W
        Ws.append(W)
        c2s.append(c2)
        prev_s1 = s1
    Wl_f = prev_s1[:, None] * np.asarray(Wl, np.float32)
    bl_f = float(np.asarray(bl, np.float32).reshape(-1)[0])

    per_core, shared, pieces, NPOS, L = _preprocess(
        x, edge_index, batch, Ws, c2s, Wl_f, bl_f)

    _LAST_RESULTS["meta"] = (pieces, NPOS, L)
    nc = _build_bass(pieces, NPOS,
                     no_collectives=bool(int(os.environ.get("GCN_NO_CC", "0"))))

    in_maps = []
    for c in range(P):
        d = dict(per_core[c])
        m = {
            "xT": np.asarray(d["xT"]), "dinv": d["dinv"], "idx": d["idx"],
            "sel": d["sel"], "pool": d["pool"],
            "W": shared["W"], "c2": shared["c2"], "post": shared["post"],
            "ident": shared["ident"], "Wl": shared["Wl"], "iota": shared["iota"],
        }
        in_maps.append(m)

    trace = bool(int(os.environ.get("GCN_TRACE", "0")))
    res = run_bass_kernel_spmd(nc, in_maps, core_ids=list(range(P)), trace=trace)
    _LAST_RESULTS["res"] = res
    out = res.results[0]["out"].reshape(G, 1).astype(np.float32)
    return out



# revision 8
# speedup vs baseline: 1.2897x; 1.2897x over previous
"""GCN (3x GCNConv + BN + ReLU, global mean pool, linear) on 8 Trainium2 cores.

Self-contained: hardcodes all shapes. Strategy:
  - Nodes block-sharded across 8 cores (12500 each); edges partitioned by dst.
  - Per layer: local GEMM h@W (bf16 on PE), row-scale by deg^-1/2, AllGather the
    bf16 node table in 4 node-slices (so gather indices fit int16), then each
    core dma_gathers its edges' source rows and segment-sums them with 0/1
    fp8 selector matmuls into fp32 PSUM.
  - Pieces are BLOCK-major: all 4 slices' groups for a dst block accumulate
    into one PSUM tile (single start/stop chain), epilogue reads PSUM directly.
  - Selectors are stored pre-packed [128, NPOS/128, 128] so their DMA runs at
    full rate (>=512B contiguous per partition).
  - BatchNorm(+bias)+ReLU folded into per-feature constants on host; ReLU on
    the scalar (ACT) engine; per-node deg^-1/2 via tensor_scalar ops.
  - Pooling: one-hot (fp16 iota compare) matmul accumulates [128f, 512g]
    partial sums, AllReduce across cores, final linear on device.
"""
import os
import numpy as np
import ml_dtypes

F = 128
P = 8
POS_BUDGET = int(os.environ.get("GCN_POS_BUDGET", "18432"))
ROUND = int(os.environ.get("GCN_ROUND", "32"))
EPS = np.float32(1e-5)


def _set_sizes(n, e, g):
    global N, E, G, NSH, NB, NPAD, TPS, SLICE_TILES, SLICE_ROWS, TBL_ROWS
    N, E, G = n, e, g
    NSH = N // P
    NB = -(-NSH // 128)
    NPAD = NB * 128
    TPS = -(-NB // 4)
    SLICE_TILES = [TPS, TPS, TPS, NB - 3 * TPS]
    assert SLICE_TILES[3] > 0
    SLICE_ROWS = [t * 128 for t in SLICE_TILES]
    TBL_ROWS = [P * r for r in SLICE_ROWS]
    assert max(TBL_ROWS) < 32768, "gather idx must fit int16"


_set_sizes(100000, 1600000, 512)

_MAXK = {0: 128, 32: 32, 64: 64}

_LAST_RESULTS = {}  # stash for test harness (exec time etc.)


def _group_segs(q0, r):
    """Segments for a group of r rows starting at in-piece position q0.
    Returns (segs, q_end) with segs = [(col, off, K), ...]."""
    q = q0
    segs = []
    while r > 0:
        off = q % 128
        K = min(r, _MAXK[off], 128 - off)
        segs.append((q // 128, off, K))
        q += K
        r -= K
    return segs, q


def _build_schedule(L):
    """L: [4][NB] int array of ROUND-multiple group lengths (same on all cores).

    Block-major pieces: each piece covers blocks [b0,b1) and holds 4 position
    sub-spaces (one per table slice k). Returns (pieces, NPOS).
    pieces: list of dicts:
      b0, b1
      k_pos0[k]: global position offset of the (piece, k) sub-space
      k_npos[k]: padded sub-space size (multiple of 128)
      blocks: list of (b, segs) with segs = [(k, col, off, K), ...] where col
              is relative to the (piece, k) sub-space.
    gstart[k][b]: in-sub-space start position of group (k, b).
    """
    # variable-size pieces: pack blocks until the 4-slice position total hits
    # POS_BUDGET, so stream tiles (sized by the largest piece) stay uniform.
    piece_spans = []
    b0 = 0
    while b0 < NB:
        b1 = b0 + 1
        tot = int(sum(L[k][b0] for k in range(4)))
        while b1 < NB:
            nxt = int(sum(L[k][b1] for k in range(4)))
            if tot + nxt > POS_BUDGET:
                break
            tot += nxt
            b1 += 1
        piece_spans.append((b0, b1))
        b0 = b1
    pieces = []
    gstart = np.zeros((4, NB), np.int64)
    pos = 0
    for (b0, b1) in piece_spans:
        k_pos0 = []
        k_npos = []
        blk_segs = {b: [] for b in range(b0, b1)}
        for k in range(4):
            q = 0
            for b in range(b0, b1):
                r = int(L[k][b])
                if r == 0:
                    continue
                if q % 128 == 96:
                    q += 32  # group starts at partition 96 are illegal
                gstart[k][b] = q
                segs, q = _group_segs(q, r)
                blk_segs[b].extend((k, c, o, kk) for (c, o, kk) in segs)
            npos = (q + 127) // 128 * 128
            k_pos0.append(pos)
            k_npos.append(npos)
            pos += npos
        blocks = [(b, blk_segs[b]) for b in range(b0, b1) if blk_segs[b]]
        pieces.append(dict(b0=b0, b1=b1, k_pos0=k_pos0, k_npos=k_npos,
                           blocks=blocks))
    return pieces, pos, gstart


def _preprocess(x, edge_index, batch, Ws, c2s, Wl, bl):
    """Build per-core device inputs. Ws: 3 pre-folded [128,128] f32 weights;
    c2s: 3 [128] f32 epilogue biases; Wl [128,1] f32; bl scalar f32."""
    src = np.asarray(edge_index[0], dtype=np.int64)
    dst = np.asarray(edge_index[1], dtype=np.int64)
    loops = np.arange(N, dtype=np.int64)
    src = np.concatenate([src, loops])
    dst = np.concatenate([dst, loops])

    deg = np.bincount(dst, minlength=N).astype(np.float32)

    # relabel nodes: sort by in-degree, deal round-robin across cores so each
    # (core, block) sees a near-identical degree profile -> the max-over-cores
    # group padding collapses toward the mean.
    order = np.argsort(-deg, kind="stable")
    perm = np.empty(N, np.int64)  # old id -> new id
    ranks = np.arange(N)
    perm[order] = (ranks % P) * NSH + ranks // P
    src = perm[src]
    dst = perm[dst]
    inv = np.empty(N, np.int64)   # new id -> old id
    inv[perm] = np.arange(N)
    x = x[inv]
    batch = np.asarray(batch, np.int64)[inv]
    deg = deg[inv]

    dinv = (1.0 / np.sqrt(np.maximum(deg, 1.0))).astype(np.float32)

    # src -> (slice k, table row)
    so = src // NSH
    si = src % NSH
    sk = np.minimum(si // SLICE_ROWS[0], 3)
    srow = so * np.array(SLICE_ROWS, np.int64)[sk] + (si - sk * SLICE_ROWS[0])
    assert srow.max() < max(TBL_ROWS)

    core = dst // NSH
    dl = dst % NSH
    db = dl // 128
    dcol = dl % 128

    # dedup: within a (core, k, block) group, a source row gathered once can
    # feed several edges via selector multiplicity. Count distinct rows.
    ekey = ((core * 4 + sk) * NB + db) * np.int64(32768) + srow
    uniq = np.unique(ekey)
    ug = uniq // 32768
    cnt = np.bincount(ug, minlength=P * 4 * NB).reshape(P, 4, NB)
    L = cnt.max(axis=0)
    L = (L + ROUND - 1) // ROUND * ROUND
    pieces, NPOS, gstart = _build_schedule(L)

    # global start position of group (k, b)
    gpos = np.zeros((4, NB), np.int64)
    for pc in pieces:
        for k in range(4):
            for b in range(pc["b0"], pc["b1"]):
                gpos[k][b] = pc["k_pos0"][k] + gstart[k][b]

    per_core = []
    for c in range(P):
        m = core == c
        skc, dbc, dcolc, srowc = sk[m], db[m], dcol[m], srow[m]
        # sort edges by (k, b, srow); dedup rows within each group
        order = np.lexsort((srowc, dbc, skc))
        skc, dbc, dcolc, srowc = (a[order] for a in (skc, dbc, dcolc, srowc))
        gid = (skc * NB + dbc) * np.int64(32768) + srowc
        first = np.r_[True, gid[1:] != gid[:-1]]          # first edge of a row
        urank = np.cumsum(first) - 1                      # dedup'd row index
        ggid = skc * NB + dbc
        gfirstmask = np.r_[True, ggid[1:] != ggid[:-1]]   # first edge of group
        # dedup'd rank within group: urank - urank[group start]
        gstart_urank = urank[gfirstmask]
        gsz = np.diff(np.r_[np.flatnonzero(gfirstmask), ggid.size])
        rank = urank - np.repeat(gstart_urank, gsz)
        posn = gpos[skc, dbc] + rank
        idx_flat = np.zeros(NPOS, np.int16)
        idx_flat[posn] = srowc.astype(np.int16)
        sel = np.zeros((NPOS, 128), np.float32)
        np.add.at(sel, (posn, dcolc), 1.0)
        assert float(sel.max()) <= 240.0
        # pack: position p -> (partition p%128, col p//128); per-partition
        # contiguous cols*128B so the DMA runs at full rate.
        sel8 = (sel.reshape(NPOS // 128, 128, 128)
                .transpose(1, 0, 2).copy().astype(ml_dtypes.float8_e4m3))
        idx_t = np.tile(idx_flat.reshape(NPOS // 16, 16).T, (8, 1)).copy()

        # node-local data
        lo = c * NSH
        xT = np.zeros((F, NPAD), ml_dtypes.bfloat16)
        xT[:, :NSH] = x[lo:lo + NSH].astype(ml_dtypes.bfloat16).T
        dv = np.zeros(NPAD, np.float32)
        dv[:NSH] = dinv[lo:lo + NSH]
        dinv_t = dv.reshape(NB, 128).T.copy()
        bv = np.full(NPAD, -1.0, np.float32)
        bv[:NSH] = np.asarray(batch[lo:lo + NSH], dtype=np.int64).astype(np.float32)
        batch_t = bv.reshape(NB, 128).T.copy()

        per_core.append(dict(
            xT=xT, dinv=dinv_t, idx=idx_t, sel=sel8, pool=batch_t,
        ))

    # shared constants
    cnt_g = np.bincount(np.asarray(batch, np.int64), minlength=G).astype(np.float32)
    invcnt = (1.0 / np.maximum(cnt_g, 1.0)).astype(np.float32)
    Wcat = np.concatenate([w.astype(np.float32) for w in Ws], axis=1).astype(ml_dtypes.bfloat16)  # [128, 384]
    c2cat = np.concatenate([np.tile(c2[None, :], (128, 1)) for c2 in c2s], axis=1).astype(np.float32)  # [128, 384]
    post = np.stack([invcnt, np.full(G, np.float32(bl))]).astype(np.float32)  # [2, 512]
    ident = np.eye(128, dtype=ml_dtypes.bfloat16)
    iota = np.tile(np.arange(G, dtype=np.float32)[None, :], (128, 1)).astype(np.float16)

    shared = dict(W=Wcat, c2=c2cat, post=post, ident=ident, iota=iota,
                  Wl=Wl.astype(np.float32).reshape(128, 1))
    return per_core, shared, pieces, NPOS, L


def _build_bass(pieces, NPOS, no_collectives=False):
    import concourse.bacc as bacc
    import concourse.mybir as mybir
    from concourse.tile import TileContext, add_dep_helper

    nc = bacc.Bacc("TRN2", target_bir_lowering=False, debug=False)
    dt = mybir.dt

    xT_in = nc.dram_tensor("xT", [F, NPAD], dt.bfloat16, kind="ExternalInput")
    dinv_in = nc.dram_tensor("dinv", [128, NB], dt.float32, kind="ExternalInput")
    idx_in = nc.dram_tensor("idx", [128, NPOS // 16], dt.int16, kind="ExternalInput")
    sel_in = nc.dram_tensor("sel", [128, NPOS // 128, 128], dt.float8e4, kind="ExternalInput")
    pool_in = nc.dram_tensor("pool", [128, NB], dt.float32, kind="ExternalInput")
    iota_in = nc.dram_tensor("iota", [128, G], dt.float16, kind="ExternalInput")
    W_in = nc.dram_tensor("W", [128, 384], dt.bfloat16, kind="ExternalInput")
    c2_in = nc.dram_tensor("c2", [128, 384], dt.float32, kind="ExternalInput")
    post_in = nc.dram_tensor("post", [2, G], dt.float32, kind="ExternalInput")
    ident_in = nc.dram_tensor("ident", [128, 128], dt.bfloat16, kind="ExternalInput")
    Wl_in = nc.dram_tensor("Wl", [128, 1], dt.float32, kind="ExternalInput")

    out_d = nc.dram_tensor("out", [1, G], dt.float32, kind="ExternalOutput")

    # internal DRAM: double-buffered per-parity cc inputs and tables
    cc_ins = [[nc.dram_tensor(f"ccin_{p}_{k}", [SLICE_ROWS[k], F], dt.bfloat16)
               for k in range(4)] for p in range(2)]
    tables = [[nc.dram_tensor(f"tbl_{p}_{k}", [TBL_ROWS[k], F], dt.bfloat16,
                              addr_space="Shared") for k in range(4)] for p in range(2)]
    ar_in = nc.dram_tensor("ar_in", [128, G], dt.float32)
    ar_out = nc.dram_tensor("ar_out", [128, G], dt.float32, addr_space="Shared")

    rg = [list(range(P))]
    maxc = max(pc["k_npos"][k] for pc in pieces for k in range(4)) // 128

    with TileContext(nc) as tc:
        with (
            tc.tile_pool(name="const", bufs=1) as cst,
            tc.tile_pool(name="big", bufs=1) as big,
            tc.tile_pool(name="io", bufs=int(os.environ.get("GCN_IO_BUFS", "3"))) as io,
            tc.tile_pool(name="stream", bufs=int(os.environ.get("GCN_STRM_BUFS", "2"))) as strm,
            tc.tile_pool(name="slab", bufs=2) as slab,
            tc.tile_pool(name="pgemm", bufs=2, space="PSUM") as pgemm,
            tc.tile_pool(name="ptrans", bufs=1, space="PSUM") as ptrans,
            tc.tile_pool(name="ppart", bufs=3, space="PSUM") as ppart,
            tc.tile_pool(name="ppool", bufs=1, space="PSUM") as ppool,
            tc.tile_pool(name="pfin", bufs=1, space="PSUM") as pfin,
        ):
            # constants
            W_sb = cst.tile([128, 384], dt.bfloat16, tag="W")
            nc.sync.dma_start(out=W_sb[:, :], in_=W_in[:, :])
            c2_sb = cst.tile([128, 384], dt.float32, tag="c2")
            nc.sync.dma_start(out=c2_sb[:, :], in_=c2_in[:, :])
            dinv_sb = cst.tile([128, NB], dt.float32, tag="dinv")
            nc.sync.dma_start(out=dinv_sb[:, :], in_=dinv_in[:, :])
            ident_sb = cst.tile([128, 128], dt.bfloat16, tag="ident")
            nc.sync.dma_start(out=ident_sb[:, :], in_=ident_in[:, :])
            Wl_sb = cst.tile([128, 1], dt.float32, tag="Wl")
            nc.sync.dma_start(out=Wl_sb[:, :], in_=Wl_in[:, :])
            iota_sb = cst.tile([128, G], dt.float16, tag="iota")
            nc.sync.dma_start(out=iota_sb[:, :], in_=iota_in[:, :])
            batch_sb = cst.tile([128, NB], dt.float32, tag="batchv")
            nc.sync.dma_start(out=batch_sb[:, :], in_=pool_in[:, :])
            invcnt_sb = cst.tile([1, G], dt.float32, tag="invcnt")
            nc.sync.dma_start(out=invcnt_sb[:, :], in_=post_in[0:1, :])
            blrow_sb = cst.tile([1, G], dt.float32, tag="blrow")
            nc.sync.dma_start(out=blrow_sb[:, :], in_=post_in[1:2, :])

            z_prev = None
            ag_by_parity = {}        # parity -> {k: ag inst}
            gathers_by_parity = {}   # parity -> {k: [gather insts]}
            poolT_ps = ppool.tile([128, G], dt.float32, tag="poolT")

            for l in range(3):
                par = l % 2
                # ---------------- GEMM phase ----------------
                ag_insts = {}
                for k in range(4):
                    roff0 = k * TPS * 128
                    nrows = SLICE_ROWS[k]
                    if l == 0:
                        xsl = slab.tile([128, SLICE_ROWS[0]], dt.bfloat16, tag="xsl")
                        nc.sync.dma_start(out=xsl[:, :nrows],
                                          in_=xT_in[:, roff0:roff0 + nrows])
                    hp = slab.tile([128, SLICE_ROWS[0]], dt.bfloat16, tag="hp")
                    for bb in range(SLICE_TILES[k]):
                        b = k * TPS + bb
                        if l == 0:
                            lhsT = xsl[:, bb * 128:(bb + 1) * 128]
                        else:
                            zT_ps = ptrans.tile([128, 128], dt.bfloat16, tag="zT")
                            nc.tensor.transpose(zT_ps[:, :], z_prev[:, b * 128:(b + 1) * 128], ident_sb[:, :])
                            lt = io.tile([128, 128], dt.bfloat16, tag="lhsT")
                            nc.vector.tensor_copy(lt[:, :], zT_ps[:, :])
                            lhsT = lt[:, :]
                        t_ps = pgemm.tile([128, 128], dt.float32, tag="t")
                        nc.tensor.matmul(t_ps[:, :], lhsT=lhsT, rhs=W_sb[:, l * 128:(l + 1) * 128],
                                         start=True, stop=True)
                        nc.vector.tensor_scalar_mul(hp[:, bb * 128:(bb + 1) * 128],
                                                    t_ps[:, :], dinv_sb[:, b:b + 1])
                    d = nc.sync.dma_start(
                        out=cc_ins[par][k].ap().rearrange("(c p) d -> p c d", p=128),
                        in_=hp[:, :nrows].rearrange("p (c d) -> p c d", d=128))
                    if par in ag_by_parity:
                        add_dep_helper(d.ins, ag_by_parity[par][k].ins,
                                       reason="ccin WAR vs prev AG same parity")
                    if no_collectives:
                        # timing stand-in: local copy of own shard
                        ag = nc.sync.dma_start(
                            out=tables[par][k][0:SLICE_ROWS[k], :],
                            in_=cc_ins[par][k][:, :])
                    else:
                        ag = nc.gpsimd.collective_compute(
                            "AllGather", mybir.AluOpType.bypass, replica_groups=rg,
                            ins=[cc_ins[par][k].ap().opt()],
                            outs=[tables[par][k].ap().opt()],
                        )
                    # WAR: table written by this AG was read by gathers 2 layers ago
                    for gi in gathers_by_parity.get(par, {}).get(k, []):
                        add_dep_helper(ag.ins, gi.ins, reason="table WAR vs old gathers")
                    ag_insts[k] = ag
                ag_by_parity[par] = ag_insts
                gathers_by_parity[par] = {k: [] for k in range(4)}

                # ---------------- SpMM phase ----------------
                z = big.tile([128, NPAD], dt.bfloat16, tag="z")
                for pc in pieces:
                    msgt = [None] * 4
                    selt = [None] * 4
                    for k in range(4):
                        npos = pc["k_npos"][k]
                        if npos == 0:
                            continue
                        pos0 = pc["k_pos0"][k]
                        cols = npos // 128
                        idxt = strm.tile([128, maxc * 8], dt.int16, tag=f"idx{k}")
                        nc.sync.dma_start(out=idxt[:, :npos // 16],
                                          in_=idx_in[:, pos0 // 16:(pos0 + npos) // 16])
                        mt = strm.tile([128, maxc, 128], dt.bfloat16, tag=f"msg{k}")
                        g = nc.gpsimd.dma_gather(
                            mt[:, :cols, :], tables[par][k][:, :], idxt[:, :npos // 16],
                            npos, npos, F,
                        )
                        add_dep_helper(g.ins, ag_insts[k].ins, reason="gather RAW on AG")
                        gathers_by_parity[par][k].append(g)
                        st = strm.tile([128, maxc, 128], dt.float8e4, tag=f"sel{k}")
                        nc.sync.dma_start(
                            out=st[:, :cols, :],
                            in_=sel_in[:, pos0 // 128:(pos0 + npos) // 128, :],
                        )
                        msgt[k] = mt
                        selt[k] = st
                    for b, segs in pc["blocks"]:
                        ps = ppart.tile([128, 128], dt.float32, tag="part")
                        for i, (k, col, off, K) in enumerate(segs):
                            nc.tensor.matmul(
                                ps[:, :],
                                lhsT=selt[k][off:off + K, col, :],
                                rhs=msgt[k][off:off + K, col, :],
                                start=(i == 0), stop=(i == len(segs) - 1),
                            )
                        # ---------------- epilogue ----------------
                        v = io.tile([128, 128], dt.float32, tag="v")
                        nc.vector.scalar_tensor_tensor(
                            v[:, :], ps[:, :], dinv_sb[:, b:b + 1],
                            c2_sb[:, l * 128:(l + 1) * 128],
                            op0=mybir.AluOpType.mult, op1=mybir.AluOpType.add,
                        )
                        zsl = z[:, b * 128:(b + 1) * 128]
                        nc.scalar.activation(zsl, v[:, :], mybir.ActivationFunctionType.Relu)
                        if l == 2:
                            pst = strm.tile([128, G], dt.float16, tag="poolsel")
                            nc.vector.tensor_scalar(
                                pst[:, :], iota_sb[:, :], batch_sb[:, b:b + 1], None,
                                op0=mybir.AluOpType.is_equal,
                            )
                            nc.tensor.matmul(poolT_ps[:, :], lhsT=zsl, rhs=pst[:, :],
                                             start=(b == 0), stop=(b == NB - 1))
                z_prev = z

            # ---------------- pooling reduce + final linear ----------------
            poolT_sb = cst.tile([128, G], dt.float32, tag="poolTsb")
            nc.vector.tensor_copy(poolT_sb[:, :], poolT_ps[:, :])
            nc.sync.dma_start(out=ar_in[:, :], in_=poolT_sb[:, :])
            if no_collectives:
                ar = nc.sync.dma_start(out=ar_out[:, :], in_=ar_in[:, :])
            else:
                ar = nc.gpsimd.collective_compute(
                    "AllReduce", mybir.AluOpType.add, replica_groups=rg,
                    ins=[ar_in.ap().opt()], outs=[ar_out.ap().opt()],
                )
            poolF = cst.tile([128, G], dt.float32, tag="poolF")
            d = nc.sync.dma_start(out=poolF[:, :], in_=ar_out[:, :])
            add_dep_helper(d.ins, ar.ins, reason="read AR output")
            out_ps = pfin.tile([1, G], dt.float32, tag="fin")
            nc.tensor.matmul(out_ps[:, :], lhsT=Wl_sb[:, :], rhs=poolF[:, :],
                             start=True, stop=True)
            orow = cst.tile([1, G], dt.float32, tag="orow")
            nc.vector.tensor_mul(orow[:, :], out_ps[:, :], invcnt_sb[:, :])
            nc.vector.tensor_add(orow[:, :], orow[:, :], blrow_sb[:, :])
            nc.sync.dma_start(out=out_d[:, :], in_=orow[:, :])

    nc.compile()
    return nc


def kernel(x, edge_index, batch,
           W1, b1, g1, be1, m1, v1,
           W2, b2, g2, be2, m2, v2,
           W3, b3, g3, be3, m3, v3,
           Wl, bl):
    from concourse.bass_utils import run_bass_kernel_spmd

    x = np.asarray(x, np.float32)
    # fold BN into per-feature scale s1 (>0) and epilogue bias c2
    Ws, c2s = [], []
    prev_s1 = None
    for (W, b, g, be, m, v) in [(W1, b1, g1, be1, m1, v1),
                                (W2, b2, g2, be2, m2, v2),
                                (W3, b3, g3, be3, m3, v3)]:
        W = np.asarray(W, np.float32)
        b = np.asarray(b, np.float32)
        g = np.asarray(g, np.float32)
        be = np.asarray(be, np.float32)
        m = np.asarray(m, np.float32)
        v = np.asarray(v, np.float32)
        s1 = g / np.sqrt(v + EPS)
        assert np.all(s1 > 0), "BN scale must be positive for ReLU folding"
        s2 = be - m * s1
        c2 = b + s2 / s1
        if prev_s1 is not None:
            W = prev_s1[:, None] * W
        Ws.append(W)
        c2s.append(c2)
        prev_s1 = s1
    Wl_f = prev_s1[:, None] * np.asarray(Wl, np.float32)
    bl_f = float(np.asarray(bl, np.float32).reshape(-1)[0])

    per_core, shared, pieces, NPOS, L = _preprocess(
        x, edge_index, batch, Ws, c2s, Wl_f, bl_f)

    _LAST_RESULTS["meta"] = (pieces, NPOS, L)
    nc = _build_bass(pieces, NPOS,
                     no_collectives=bool(int(os.environ.get("GCN_NO_CC", "0"))))

    in_maps = []
    for c in range(P):
        d = dict(per_core[c])
        m = {
            "xT": np.asarray(d["xT"]), "dinv": d["dinv"], "idx": d["idx"],
            "sel": d["sel"], "pool": d["pool"],
            "W": shared["W"], "c2": shared["c2"], "post": shared["post"],
            "ident": shared["ident"], "Wl": shared["Wl"], "iota": shared["iota"],
        }
        in_maps.append(m)

    trace = bool(int(os.environ.get("GCN_TRACE", "0")))
    res = run_bass_kernel_spmd(nc, in_maps, core_ids=list(range(P)), trace=trace)
    _LAST_RESULTS["res"] = res
    out = res.results[0]["out"].reshape(G, 1).astype(np.float32)
    return out
